# revision 35
# baseline (speedup 1.0000x reference)
"""Trainium2 Bass kernel for a single-head linear-projection attention block.

Reference computation (B=4, CH=256, N=4096):
    theta = Wt @ x        [B, 32, N]
    phi   = Wp @ x        [B, 32, N]
    g     = Wg @ x        [B, 128, N]
    scores = theta^T phi  [B, N, N]
    beta = softmax(scores, axis=-1)
    attn = g @ beta^T     [B, 128, N]
    out = gamma * (Wo @ attn) + x

Sharding: 8 cores = 4 batches x 2 query-halves. Each core owns one batch's
full sequence (keys/values) and half the queries; per-core x is rotated so
its query half is always columns 0:2048 (softmax/attention are invariant to
a consistent permutation of the key axis). No collectives.

Per-core dataflow (all matmuls bf16, fp32 PSUM). The kernel is a flat
stream of 64 tiles (16 m-tile pairs x 4 query passes); for tile t:
  - step t-1: score matmuls (scoresT[m,n], two concurrent K=32 row-group
    matmuls; theta/phi replicated to two 32-partition groups)
  - step t:   exp extraction of the PSUM scores, split across TWO engines:
    most tiles via ScalarE ACTIVATE(Exp), some via VectorE with a
    Schraudolph bit-trick (i16 = s*a + b bitcast to bf16 approximates exp;
    softmax averaging suppresses the +-3% ripple to ~1e-3 output error)
  - step t+1: attention matmuls + denominator partial (PE ones-matmuls
    for some tiles, a bf16 pairwise-add tree on VectorE for the rest)
The two-deep skew means every PE instruction's inputs are complete when it
reaches the strict-FIFO PE queue head: the steady-state period is
max(engine busy), not the exp->attn->scores dependency chain.

Prologue is DMA-bound (~70 GB/s per queue): all four projection weights
arrive host-packed in ONE [128, 640] tensor (tiny-row weight DMAs cost
~7us otherwise); x streams in 1024-column chunks round-robin over the
three DMA-capable queues (sync/scalar/gpsimd) in consumption order; a
small cols-0:1024 theta/phi replication unblocks the first score matmuls
while the full replication finishes behind it. Exp LUT preloaded; a dummy
matmul burst trips the PE clock monitor (HAM) to 2.4 GHz. Epilogue per
pass (reciprocal, normalize, Wo projection, bf16 residual add) is dripped
into the next pass's steps.
"""

import os
import sys

import numpy as np

B, CH, N = 4, 256, 4096
NCORES = 8
NH = N // 2  # queries per core
P = 128

_REPO_CANDIDATES = ["/opt/trn_rl_repo", "/root/.axon_site/_ro/trn_rl_repo"]


def _ensure_import_path():
    try:
        import concourse.bass  # noqa: F401
        return
    except ImportError:
        pass
    for cand in _REPO_CANDIDATES:
        if os.path.isdir(cand):
            sys.path.insert(0, cand)
            try:
                import concourse.bass  # noqa: F401
                return
            except ImportError:
                sys.path.pop(0)
    raise ImportError("could not locate concourse (bass) repo")


_CACHE = {}

MT = N // P    # 32 m-tiles
NQ = 512       # query chunk per pass (4 passes over n)
NHALF = MT     # 32 half-steps (one m-tile each) per pass
NSTEPS = 4 * NHALF

# ---- tuning knobs ----
# pass-local half-steps whose exp extraction runs on VectorE (Schraudolph
# bit trick); the rest are ScalarE ACTIVATE(Exp). With five single-bank
# PSUM score slots both engines extract CONCURRENTLY.
H_V = frozenset({3, 7, 11, 15, 18, 21, 24, 27, 30})
# half-steps whose denominator partial runs as PE ones-matmuls; must start
# at >=12 so the shared ones/Wo PSUM bank is freed by the dripped epilogue
R_HALF = frozenset(range(20, 32))
R_LAST = frozenset(range(20, 32))
# pass 0 carries the background projection/gT work on the PE, and its
# VectorE has no epilogue drips to run: shift most of its denominator
# reduction to the tree
R_FIRST = frozenset(range(26, 32))
# lvl0 tree adds per pass offloaded to GpSimd
G_ADDS = 4
# epilogue drip positions (pass-local half-steps) within the next pass
DRIP_HH = (2, 3, 4, 5, 6, 7)
# Schraudolph constants: i16 = s*SCH_A + SCH_B, bitcast int16->bf16
_LOG2E = 1.4426950408889634
SCH_A = 128.0 * _LOG2E
SCH_B = 127.0 * 128.0 - 0.043677 * 128.0


def build_bass():
    """Build + compile the per-core Tile program (identical on all 8 cores)."""
    _ensure_import_path()
    import concourse.bacc as bacc
    import concourse.tile as tile
    from concourse import mybir

    dt = mybir.dt
    f32 = dt.float32
    bf16 = dt.bfloat16
    i16 = dt.int16
    Exp = mybir.ActivationFunctionType.Exp
    Mult = mybir.AluOpType.mult
    Add = mybir.AluOpType.add

    nc = bacc.Bacc(
        "TRN2",
        target_bir_lowering=False,
        debug=False,
        num_devices=NCORES,
    )

    # Per-core DRAM I/O. All projection weights arrive host-packed in one
    # [128, 640] bf16 tensor (1280B rows; separate tensors would DMA as
    # 64B descriptors at ~5 GB/s):
    #   cols 0:64    Wt^T   as [p, kb*32+m]
    #   cols 64:128  Wp^T   as [p, kb*32+m]
    #   cols 128:384 Wg^T   as [p, kb*128+m]
    #   cols 384:640 (gamma*Wo)^T
    x_d = nc.dram_tensor("x", [CH, N], bf16, kind="ExternalInput")
    w_d = nc.dram_tensor("w", [P, 640], bf16, kind="ExternalInput")
    out_d = nc.dram_tensor("out", [CH, NH], bf16, kind="ExternalOutput")

    with tile.TileContext(nc) as tc:
        with (
            tc.tile_pool(name="const", bufs=1) as const,
            tc.tile_pool(name="xp", bufs=1) as xp,
            tc.tile_pool(name="proj", bufs=1) as proj,
            tc.tile_pool(name="expp", bufs=16) as expp,
            tc.tile_pool(name="acc", bufs=2) as acc,
            tc.tile_pool(name="outp", bufs=1) as outp,
            tc.tile_pool(name="tree", bufs=3) as tree,
            tc.tile_pool(name="ps2", bufs=5, space="PSUM") as ps2,
            tc.tile_pool(name="psA", bufs=2, space="PSUM") as psA,
            tc.tile_pool(name="psO", bufs=1, space="PSUM") as psO,
        ):
            w_sb = const.tile([P, 640], bf16)
            ones_sb = const.tile([P, P], bf16)
            nc.scalar.dma_start(out=w_sb, in_=w_d.ap())

            def wt(kb):
                return w_sb[:, kb * 32:(kb + 1) * 32]

            def wp(kb):
                return w_sb[:, 64 + kb * 32:64 + (kb + 1) * 32]

            def wg(kb):
                return w_sb[:, 128 + kb * P:128 + (kb + 1) * P]

            wo_sb = w_sb[:, 384:640]

            # ---- x: 256KB calls with 2KB rows, round-robin over the three
            # DMA queues, in consumption (column) order ----
            x_sb = xp.tile([P, 2, N], bf16)
            qs = (nc.sync, nc.gpsimd, nc.scalar)
            qi = 0
            for cb in range(4):
                for kb in range(2):
                    qs[qi % 3].dma_start(
                        out=x_sb[:, kb, cb * 1024:(cb + 1) * 1024],
                        in_=x_d[kb * P:(kb + 1) * P, cb * 1024:(cb + 1) * 1024],
                    )
                    qi += 1

            nc.vector.memset(ones_sb, 1.0)

            # preload the exp LUT during the x-DMA wait (first use is ~2.7us)
            warm_act = const.tile([P, 8], bf16)
            nc.scalar.activation(out=warm_act, in_=ones_sb[:, 0:8], func=Exp)

            # dummy matmul burst during the initial x-DMA wait: trips the PE
            # clock monitor (HAM, needs >3.4us sustained) to full rate
            warm_sb = const.tile([P, 512], bf16)
            nc.vector.memset(warm_sb, 0.0)
            for _ in range(15):
                ps_w = ps2.tile([P, 512], f32, tag="ps")
                nc.tensor.matmul(ps_w, lhsT=ones_sb, rhs=warm_sb, start=True, stop=True)

            def split_copy(dst_s, src_s, dst_v, src_v):
                """PSUM->SBUF copy split across ScalarE and VectorE."""
                nc.scalar.copy(out=dst_s, in_=src_s)
                nc.vector.tensor_copy(out=dst_v, in_=src_v)

            # ---- projections: theta natively at partitions 0:32 (col group
            # 0), phi at 32:64 (col group 1); theta replicated to 32:64 and
            # phi to 0:32 for 2-way row-group score packing. A small
            # cols-0:1024 replication goes first so pass 0 can start. ----
            th_rep = proj.tile([P, NH], bf16)
            ph_rep = proj.tile([P, N], bf16)
            gT_sb = proj.tile([P, MT, P], bf16)

            def emit_proj_cb(cb):
                cbs = slice(cb * 1024, (cb + 1) * 1024)
                pp = []
                for c in range(2):
                    ps_p = ps2.tile([64, 512], f32, tag="ps")
                    pp.append(ps_p)
                    for kb in range(2):
                        if cb < 2:
                            nc.tensor.matmul(
                                ps_p[0:32, :],
                                lhsT=wt(kb),
                                rhs=x_sb[:, kb, cb * 1024 + c * 512:cb * 1024 + (c + 1) * 512],
                                start=(kb == 0),
                                stop=(kb == 1),
                                skip_group_check=True,
                            )
                        nc.tensor.matmul(
                            ps_p[32:64, :],
                            lhsT=wp(kb),
                            rhs=x_sb[:, kb, cb * 1024 + c * 512:cb * 1024 + (c + 1) * 512],
                            start=(kb == 0),
                            stop=(kb == 1),
                            skip_group_check=True,
                        )
                h0 = slice(cb * 1024, cb * 1024 + 512)
                h1 = slice(cb * 1024 + 512, cb * 1024 + 1024)
                if cb < 2:
                    split_copy(th_rep[0:32, h0], pp[0][0:32, :],
                               th_rep[0:32, h1], pp[1][0:32, :])
                split_copy(ph_rep[32:64, h0], pp[0][32:64, :],
                           ph_rep[32:64, h1], pp[1][32:64, :])
                if cb == 0:
                    # unblock pass 0 iters 0..3: replicate theta's first
                    # pass slice and phi's first chunk right away
                    nc.gpsimd.dma_start(
                        out=th_rep[32:64, 0:NQ], in_=th_rep[0:32, 0:NQ]
                    )
                    nc.sync.dma_start(
                        out=ph_rep[0:32, 0:1024], in_=ph_rep[32:64, 0:1024]
                    )
                else:
                    nc.sync.dma_start(
                        out=ph_rep[0:32, cbs], in_=ph_rep[32:64, cbs]
                    )
                if cb == 1:
                    nc.gpsimd.dma_start(
                        out=th_rep[32:64, NQ:NH], in_=th_rep[0:32, NQ:NH]
                    )

            # ---- gT: x-stationary matmuls (g transposed, m on partitions);
            # the dense burst also keeps the PE clock (HAM) warm ----
            def emit_gt_group(grp):
                """4 m-tiles per group (one PSUM bank), 8 groups total."""
                ps_g = ps2.tile([P, 4, P], f32, tag="ps")
                for j in range(4):
                    mt = grp * 4 + j
                    for kb in range(2):
                        nc.tensor.matmul(
                            ps_g[:, j, :],
                            lhsT=x_sb[:, kb, mt * P:(mt + 1) * P],
                            rhs=wg(kb),
                            start=(kb == 0),
                            stop=(kb == 1),
                        )
                split_copy(gT_sb[:, grp * 4:grp * 4 + 2, :], ps_g[:, 0:2, :],
                           gT_sb[:, grp * 4 + 2:grp * 4 + 4, :], ps_g[:, 2:4, :])

            emit_proj_cb(0)
            emit_proj_cb(1)
            emit_gt_group(0)
            emit_gt_group(1)

            out_sb = outp.tile([P, 2, NH], bf16)

            def emit_scores(h):
                """One K=32 score matmul for half-tile h; adjacent halves
                use the two row groups so paired emissions run concurrently
                in the PE array."""
                nh, mt = divmod(h, NHALF)
                ns = slice(nh * NQ, (nh + 1) * NQ)
                j = mt % 2
                ps_s = ps2.tile([P, NQ], f32, tag="ps")
                nc.tensor.matmul(
                    ps_s,
                    lhsT=ph_rep[32 * j:32 * (j + 1), mt * P:(mt + 1) * P],
                    rhs=th_rep[32 * j:32 * (j + 1), ns],
                    start=True,
                    stop=True,
                    skip_group_check=True,
                )
                return ps_s

            def epilogue_pieces(nh, attn_ps, ones_ps):
                """Dripped into the next pass. Piece 0 frees the shared
                ones/Wo PSUM bank; the Wo projections then reuse it, and the
                next pass's first ones-matmul (half-step >=12) follows."""
                recip = acc.tile([P, NQ], f32, tag="recip")
                nc.vector.reciprocal_approx_fast(out=recip, in_=ones_ps)
                yield
                A_bf = acc.tile([P, NQ], bf16, tag="abf")
                nc.vector.tensor_mul(A_bf, attn_ps, recip)
                yield
                sl = slice(nh * NQ, (nh + 1) * NQ)
                ps_o0 = psO.tile([P, NQ], f32, tag="ones")
                nc.tensor.matmul(
                    ps_o0, lhsT=wo_sb[:, 0:P], rhs=A_bf, start=True, stop=True,
                )
                yield
                nc.vector.tensor_add(out_sb[:, 0, sl], ps_o0, x_sb[:, 0, sl])
                if nh == 3:
                    m = nh * NQ + NQ // 2
                    nc.sync.dma_start(out=out_d[0:P, nh * NQ:m],
                                      in_=out_sb[:, 0, nh * NQ:m])
                    nc.gpsimd.dma_start(out=out_d[0:P, m:(nh + 1) * NQ],
                                        in_=out_sb[:, 0, m:(nh + 1) * NQ])
                else:
                    oq = (nc.sync, nc.gpsimd)[nh % 2]
                    oq.dma_start(out=out_d[0:P, sl], in_=out_sb[:, 0, sl])
                yield
                ps_o1 = psO.tile([P, NQ], f32, tag="ones")
                nc.tensor.matmul(
                    ps_o1, lhsT=wo_sb[:, P:CH], rhs=A_bf, start=True, stop=True,
                )
                yield
                nc.vector.tensor_add(out_sb[:, 1, sl], ps_o1, x_sb[:, 1, sl])
                if nh == 3:
                    m = nh * NQ + NQ // 2
                    nc.sync.dma_start(out=out_d[P:CH, nh * NQ:m],
                                      in_=out_sb[:, 1, nh * NQ:m])
                    nc.gpsimd.dma_start(out=out_d[P:CH, m:(nh + 1) * NQ],
                                        in_=out_sb[:, 1, m:(nh + 1) * NQ])
                else:
                    oq2 = (nc.gpsimd, nc.sync)[nh % 2]
                    oq2.dma_start(out=out_d[P:CH, sl], in_=out_sb[:, 1, sl])
                yield

            # ---- per-pass reduction/accumulation state ----
            state = {}

            def consume(h, expt):
                """Attention matmul + denominator partial for half-tile h
                (runs two half-steps after h's extraction)."""
                nh, mt = divmod(h, NHALF)
                if mt == 0:
                    attn_ps = psA.tile([P, NQ], f32, tag="attn")
                    state.clear()
                    state.update(attn=attn_ps, quads={}, chain=None,
                                 ones=None, nh=nh, gp=0)
                nc.tensor.matmul(
                    state["attn"],
                    lhsT=gT_sb[:, mt, :],
                    rhs=expt,
                    start=(mt == 0),
                    stop=(mt == NHALF - 1),
                    skip_group_check=True,
                )
                if mt in (R_FIRST if nh == 0 else (R_HALF if nh < 3 else R_LAST)):
                    if state["ones"] is None:
                        state["ones"] = psO.tile([P, NQ], f32, tag="ones", name="ones_acc")
                        state["ones_started"] = False
                    nc.tensor.matmul(
                        state["ones"],
                        lhsT=ones_sb,
                        rhs=expt,
                        start=not state["ones_started"],
                        stop=False,
                        skip_group_check=True,
                    )
                    state["ones_started"] = True
                else:
                    quads = state["quads"]
                    node, lvl = expt, 0
                    while lvl in quads and lvl < 2:
                        prev = quads.pop(lvl)
                        nt = tree.tile([P, NQ], bf16, tag=f"tree_l{lvl}")
                        if lvl == 0 and state["gp"] < G_ADDS:
                            nc.gpsimd.tensor_add(nt, prev, node)
                            state["gp"] += 1
                        else:
                            nc.vector.tensor_add(nt, prev, node)
                        node, lvl = nt, lvl + 1
                    if lvl < 2:
                        quads[lvl] = node
                    elif state["chain"] is None:
                        state["chain"] = node
                    else:
                        nt = tree.tile([P, NQ], bf16, tag="chain")
                        nc.vector.tensor_add(nt, state["chain"], node)
                        state["chain"] = nt

            def pass_tail():
                """Finish a pass: merge tree leftovers, one final
                partition-reducing ones-matmul; spawn the epilogue."""
                chain = state["chain"]
                for lv in sorted(state["quads"]):
                    node = state["quads"].pop(lv)
                    if chain is None:
                        chain = node
                    else:
                        nt = tree.tile([P, NQ], bf16, tag="chain")
                        nc.vector.tensor_add(nt, chain, node)
                        chain = nt
                nc.tensor.matmul(
                    state["ones"], lhsT=ones_sb, rhs=chain,
                    start=False, stop=True, skip_group_check=True,
                )
                return epilogue_pieces(state["nh"], state["attn"], state["ones"])

            # ---- flat 128-half-tile stream, two-deep software pipeline;
            # scores emitted in row-group pairs so they pack in the PE ----
            pending = None
            exp_by_h = {}
            ps_by_h = {0: emit_scores(0), 1: emit_scores(1)}
            for h in range(NSTEPS):
                hh = h % NHALF
                # 1. scores two halves ahead, emitted in adjacent pairs
                if h % 2 == 0:
                    if h + 2 < NSTEPS:
                        ps_by_h[h + 2] = emit_scores(h + 2)
                    if h + 3 < NSTEPS:
                        ps_by_h[h + 3] = emit_scores(h + 3)
                # 2. exp extraction of half-tile h (frees its score slot)
                ps_cur = ps_by_h.pop(h)
                expt = expp.tile([P, NQ], bf16, tag="expt")
                if hh in H_V:
                    nc.vector.tensor_scalar(
                        out=expt.bitcast(i16),
                        in0=ps_cur,
                        scalar1=SCH_A,
                        scalar2=SCH_B,
                        op0=Mult,
                        op1=Add,
                    )
                else:
                    nc.scalar.activation(out=expt, in_=ps_cur, func=Exp)
                exp_by_h[h] = expt
                # 3. attention + reduction for half-tile h-2
                if h >= 2:
                    consume(h - 2, exp_by_h.pop(h - 2))
                    if (h - 2) % NHALF == NHALF - 1:
                        pending = pass_tail()
                # 4. background prologue work: remaining projections,
                # replication and gT groups, ahead of their first consumers
                if h == 2:
                    emit_gt_group(2)
                elif h == 4:
                    emit_gt_group(3)
                elif h == 6:
                    emit_proj_cb(2)
                elif h == 10:
                    emit_gt_group(4)
                elif h == 12:
                    emit_gt_group(5)
                elif h == 14:
                    emit_proj_cb(3)
                elif h == 18:
                    emit_gt_group(6)
                elif h == 20:
                    emit_gt_group(7)
                # 5. drip the previous pass's epilogue
                if pending is not None and hh in DRIP_HH:
                    next(pending, None)
            consume(NSTEPS - 2, exp_by_h.pop(NSTEPS - 2))
            consume(NSTEPS - 1, exp_by_h.pop(NSTEPS - 1))
            pending2 = pass_tail()
            if pending is not None:
                for _ in pending:
                    pass
            for _ in pending2:
                pass

    nc.compile()
    return nc


def get_nc():
    if "nc" not in _CACHE:
        _CACHE["nc"] = build_bass()
    return _CACHE["nc"]


def make_in_maps(x, Wt, Wp, Wg, Wo, gamma):
    import ml_dtypes

    bf16 = ml_dtypes.bfloat16
    x = np.asarray(x, dtype=np.float32)
    wtT = np.asarray(Wt, np.float32).T   # [256, 32]
    wpT = np.asarray(Wp, np.float32).T   # [256, 32]
    wgT = np.asarray(Wg, np.float32).T   # [256, 128]
    woT = (float(np.asarray(gamma)) * np.asarray(Wo, np.float32)).T  # [128, 256]
    w_all = np.concatenate(
        [
            wtT[0:P], wtT[P:CH],
            wpT[0:P], wpT[P:CH],
            wgT[0:P], wgT[P:CH],
            woT,
        ],
        axis=1,
    )  # [128, 640]
    w_all = np.ascontiguousarray(w_all).astype(bf16)
    in_maps = []
    for i in range(NCORES):
        b, h = divmod(i, 2)
        xb = x[b]
        if h:
            xb = np.concatenate([xb[:, NH:], xb[:, :NH]], axis=1)
        in_maps.append(
            {
                "x": np.ascontiguousarray(xb).astype(bf16),
                "w": w_all,
            }
        )
    return in_maps


def gather_out(results):
    out = np.empty((B, CH, N), np.float32)
    for i in range(NCORES):
        b, h = divmod(i, 2)
        out[b][:, h * NH:(h + 1) * NH] = results[i]["out"]
    return out


def kernel(x, Wt, Wp, Wg, Wo, gamma):
    _ensure_import_path()
    from concourse.bass_utils import run_bass_kernel_spmd

    nc = get_nc()
    in_maps = make_in_maps(x, Wt, Wp, Wg, Wo, gamma)
    res = run_bass_kernel_spmd(nc, in_maps, core_ids=list(range(NCORES)))
    return gather_out(res.results)
